# revision 50
# baseline (speedup 1.0000x reference)
"""Bass/Trainium2 kernel for chunked local attention with memory tokens
(BertSelfAttention variant). Self-contained: hardcodes all shapes.

Sharding: 8 cores, each handles 4 of the 32 (batch, chunk) pairs.
  core i -> b = i // 4, chunks 4*(i % 4) .. 4*(i % 4) + 3
No collectives; weights replicated per core; host scatters/gathers.

Per-core device computation (PE operands fp16, accumulation fp32):
  - xT [128, 8, 1024] arrives pre-transposed from host (feature-major),
    sliced per chunk; no device transposes
  - qT[jt] = x@(Wq/8) + bq/8 (feature-major)   kT[jt] = x@Wk + bk
  - v[t, yt, h, 65] token-major with a ones column at 64 that makes the
    PV matmul emit softmax denominators
  - memory-token K is materialized block-diagonally (kTm3 [128, jt, 64]:
    rows 0:64 x cols 0:16 = even head, rows 64:128 x cols 32:48 = odd) so
    one 128-contraction matmul scores 2 heads' mem keys at 32-aligned
    psum partitions; all 16 heads' mem scores share one 2-bank psum and
    a single Exp
  - local scoresT per (head-quad, yt) fill a 2-bank [128, 1024] psum
    (both 64-row ab halves, quadrant-packed) -> one Exp per fill
  - probs = Exp(scoresT + mask[y]) in fp16; additive key mask rides the
    ACT bias operand (per-partition scalar)
  - out_unnorm[x, 4*(hd|den)] per head-pair in one psum bank; strided
    reciprocal + one broadcast multiply normalize and emit fp16 out
Output fp16 on device, upcast to fp32 on host.

Scheduling: generator-based software pipelining weaves chunk ci's
attention with chunk ci+1's projections (attn(ci+1) steps twice per
round so its fills/Exps run far ahead) so the PE streams matmuls while
ACT runs the Exps; tiny warm matmuls keep the PE p-state ramp clock
alive across startup DMA waits.

DMA orchestration (the cost model charges ~625ns serialized HWDGE issue
+ serial transfers on one DMA-engine set + 900ns sem per DMA):
  - Q/K weights stream on the sync queue as 256KB granules in the exact
    order chunk-0's psum fills consume them, with x(0) halves
    interleaved at first use -> first projection matmul at ~4.4us
    (DMA-pipe floor) instead of ~7.8us
  - small constants (biases, mask, mem tokens) go through the Pool
    engine's SWDGE path, off the HWDGE critical chain; the V bias
    broadcast [128, D] is a DMA (no PE/DVE work)
  - mem-V vm: bias folded into the psum as a rank-1 (bvrow^T @ ones)
    matmul, ACT copies psum->fp16 (Pool cannot touch PSUM), and the
    head-major relayout + 4 partition-base replicas come straight off
    the transposed buffer as DMAs (no DVE hop)
  - the last chunk defers unit 6's normalize past unit 7's matmuls so
    only recip7+mult7+one 256KB DMA trail the final PE instruction
"""

import sys

sys.path.insert(0, "/opt/trn_rl_repo")

import numpy as np

import concourse.tile as tile
from concourse import bacc, mybir
from concourse.bass_utils import run_bass_kernel_spmd

F32 = mybir.dt.float32
F16 = mybir.dt.float16

B, S, D = 2, 4096, 1024
H, HD = 16, 64
W = 256            # attention window (chunk length)
C = S // W         # 16 chunks
M = 16             # memory tokens
N_CORES = 8
CPC = C * B // N_CORES  # 4 chunks per core
TPC = CPC * W           # 1024 chunk tokens per core
NJT = D // 128          # 8 feature tiles
VW = HD + 1             # v width (64 hd + ones col)

TRACE = False
LAST_RESULTS = None


def _mem_slot(h):
    """(partition base, free block) of head h's mem scores / probs."""
    jt, u = h // 2, h % 2
    return 64 * (jt % 2) + 32 * u, jt // 2


def _build_kernel():
    nc = bacc.Bacc(None, target_bir_lowering=False)

    xT_d = nc.declare_dram_parameter("xT", [128, NJT * TPC], F16, isOutput=False)
    memT_d = nc.declare_dram_parameter("memT", [128, NJT * M], F16, isOutput=False)
    wqT_d = nc.declare_dram_parameter("wqT", [D, D], F16, isOutput=False)
    wkT_d = nc.declare_dram_parameter("wkT", [D, D], F16, isOutput=False)
    wvT_d = nc.declare_dram_parameter("wvT", [D, D], F16, isOutput=False)
    bq_d = nc.declare_dram_parameter("bqv", [128, NJT], F32, isOutput=False)
    bk_d = nc.declare_dram_parameter("bkv", [128, NJT], F32, isOutput=False)
    bvr_d = nc.declare_dram_parameter("bvbr", [1, D], F32, isOutput=False)
    bvr16_d = nc.declare_dram_parameter("bvbr16", [1, D], F16, isOutput=False)
    msk_d = nc.declare_dram_parameter("maskvT", [128, CPC * 3], F32, isOutput=False)
    out_d = nc.declare_dram_parameter("out", [TPC, D], F16, isOutput=True)

    with tile.TileContext(nc) as tc:
        with (
            tc.tile_pool(name="const", bufs=1) as cpool,
            tc.tile_pool(name="wpool", bufs=1) as wpool,
            tc.tile_pool(name="xtpool", bufs=4) as xtpool,
            tc.tile_pool(name="qkpool", bufs=16) as qkpool,
            tc.tile_pool(name="vpool", bufs=4) as vpool,
            tc.tile_pool(name="epool", bufs=16) as epool,
            tc.tile_pool(name="empool", bufs=3) as empool,
            tc.tile_pool(name="opool", bufs=4) as opool,
            tc.tile_pool(name="rpool", bufs=4) as rpool,
            tc.tile_pool(name="pp", bufs=2, space="PSUM") as pp_pool,
            tc.tile_pool(name="ps", bufs=2, space="PSUM") as ps_pool,
            tc.tile_pool(name="po", bufs=2, space="PSUM") as po_pool,
        ):
            x_tiles = {}

            def load_x(ci, eng, split=False):
                x_t = xtpool.tile([128, NJT, W], F16, tag="xT", name="xT")
                xs = xT_d.rearrange("p (o t) -> p o t", t=TPC)[
                    :, :, ci * W:(ci + 1) * W
                ]
                if split:
                    eng.dma_start(x_t[:, 0:4, :], xs[:, 0:4, :])
                    eng.dma_start(x_t[:, 4:8, :], xs[:, 4:8, :])
                else:
                    eng.dma_start(x_t[:], xs)
                x_tiles[ci] = [x_t[:, d, :] for d in range(NJT)]

            w_all = wpool.tile([128, 3 * NJT, D], F16, tag="w_all")

            def load_w_piece(wi, wd, o0, og, c0, cw, eng):
                eng.dma_start(
                    w_all[:, wi * NJT + o0: wi * NJT + o0 + og, c0:c0 + cw],
                    wd.rearrange("(o p) c -> p o c", p=128)[
                        :, o0:o0 + og, c0:c0 + cw
                    ],
                )

            def wq(d):
                return w_all[:, d, :]

            def wk(d):
                return w_all[:, NJT + d, :]

            def wv(d):
                return w_all[:, 2 * NJT + d, :]

            # ---- pool-queue constants (SWDGE path, bypasses the HWDGE
            # serialization on the sync queue): ones first so the PE warm
            # matmul fires ~250ns in, then small constants in consumption
            # order, then the V weights (needed only mid-chunk-0).
            ones1 = cpool.tile([1, 128], F16, tag="ones1")
            nc.gpsimd.memset(ones1[:], 1.0)
            bqv = cpool.tile([128, NJT], F32, tag="bqv")
            nc.gpsimd.dma_start(bqv[:], bq_d[:])
            bkv = cpool.tile([128, NJT], F32, tag="bkv")
            nc.gpsimd.dma_start(bkv[:], bk_d[:])
            mskv = cpool.tile([128, CPC * 3], F32, tag="mskv")
            nc.gpsimd.dma_start(mskv[:], msk_d[:])
            xTm = cpool.tile([128, NJT, M], F16, tag="xTm")
            nc.gpsimd.dma_start(xTm[:], memT_d.rearrange("p (o m) -> p o m", m=M))
            bvrow = cpool.tile([1, D], F16, tag="bvrow")
            nc.gpsimd.dma_start(bvrow[:], bvr16_d[:])
            bvb = cpool.tile([128, D], F32, tag="bvb")

            # ---- sync-queue streams: 128KB weight granules in the exact
            # order chunk-0's psum fills consume them (jp-pair cols x d-half
            # rows), with x(0) halves interleaved at their first use.
            x_t0 = xtpool.tile([128, NJT, W], F16, tag="xT", name="xT")
            xs0 = xT_d.rearrange("p (o t) -> p o t", t=TPC)[:, :, 0:W]
            nc.sync.dma_start(x_t0[:, 0:4, :], xs0[:, 0:4, :])
            x_tiles[0] = [x_t0[:, d, :] for d in range(NJT)]
            # 256KB granules in chunk-0 consumption order (jp-pair cols x
            # d-half rows)
            GRAN = ((0, 0), (0, 256), (4, 0), (4, 256),
                    (0, 512), (0, 768), (4, 512), (4, 768))
            for o0, c0 in GRAN[:2]:
                load_w_piece(0, wqT_d, o0, 4, c0, 256, nc.sync)
            nc.sync.dma_start(x_t0[:, 4:8, :], xs0[:, 4:8, :])
            for o0, c0 in GRAN[2:]:
                load_w_piece(0, wqT_d, o0, 4, c0, 256, nc.sync)
            for o0, c0 in GRAN:
                load_w_piece(1, wkT_d, o0, 4, c0, 256, nc.sync)
            load_w_piece(2, wvT_d, 0, 4, 0, D, nc.sync)
            load_w_piece(2, wvT_d, 4, 4, 0, D, nc.sync)
            nc.sync.dma_start(bvb[:], bvr_d[:].to_broadcast((128, D)))
            load_x(1, nc.sync)
            load_x(2, nc.sync)
            load_x(3, nc.sync)

            # tiny dep-free matmuls: start the PE p-state ramp clock early
            # (ones lands ~250ns via the pool memset) and refresh it off the
            # small pool DMAs so the idle gaps before the first projection
            # fills stay under the ~3us p-state reset threshold
            ps_warm = pp_pool.tile([128, 512], F32, tag="pp")
            nc.tensor.matmul(
                ps_warm[:16, :16], ones1[:, :16], ones1[:, :16],
                start=True, stop=True,
            )
            nc.tensor.matmul(
                ps_warm[:4, 16:20], bqv[:16, :4], bqv[:16, :4],
                start=True, stop=True,
            )
            x0d0 = x_tiles[0][0]
            nc.tensor.matmul(
                ps_warm[:16, 32:48], x0d0[:16, :16], x0d0[:16, :16],
                start=True, stop=True,
            )

            # ---- memory tokens (emitted inside chunk 0's flow) ----
            memp = {}

            def emit_ktm():
                # block-diagonal mem-K: [128, jt, 64]; even head rows 0:64 ->
                # cols 0:16, odd head rows 64:128 -> cols 32:48, rest zero
                kTm3 = cpool.tile([128, NJT, 64], F16, tag="kTm3", name="kTm3")
                nc.gpsimd.memset(kTm3[:], 0.0)
                ps_k = pp_pool.tile([128, 512], F32, tag="pp", name="ps_ktm")
                for jt in range(NJT):
                    for d in range(NJT):
                        nc.tensor.matmul(
                            ps_k[:, jt * M:(jt + 1) * M],
                            wk(d)[:, jt * 128:(jt + 1) * 128],
                            xTm[:, d, :],
                            start=(d == 0), stop=(d == NJT - 1),
                            skip_group_check=True,
                        )
                for u in range(2):
                    nc.vector.tensor_tensor(
                        kTm3[64 * u:64 * (u + 1), :, 32 * u:32 * u + 16],
                        ps_k[64 * u:64 * (u + 1), :NJT * M].rearrange(
                            "p (j m) -> p j m", m=M
                        ),
                        bkv[64 * u:64 * (u + 1), :][:, :, None].to_broadcast(
                            (64, NJT, M)
                        ),
                        mybir.AluOpType.add,
                    )
                memp["kTm3"] = kTm3

            def emit_vm():
                # mem-V computed feature-major in one cheap psum fill, then
                # token-major via an xbar DMA-transpose of the m-padded
                # [128, jt, 128] layout (dst[p, jt, f] = src[f, jt*128+p]);
                # replicated at partition bases 0/32/64/96 for the
                # 32-aligned mem-PV stationaries
                vm = cpool.tile([128, M, VW], F16, tag="vm", name="vm")
                for rb in range(4):
                    nc.gpsimd.memset(vm[32 * rb:32 * rb + M, :, HD:HD + 1], 1.0)
                vmT = cpool.tile([128, NJT, 128], F16, tag="vmT", name="vmT")
                nc.gpsimd.memset(vmT[:], 0.0)
                ps_t = pp_pool.tile([128, 512], F32, tag="pp", name="ps_vm")
                for jt in range(NJT):
                    for d in range(NJT):
                        nc.tensor.matmul(
                            ps_t[:, jt * M:(jt + 1) * M],
                            wv(d)[:, jt * 128:(jt + 1) * 128],
                            xTm[:, d, :],
                            start=(d == 0), stop=False,
                            skip_group_check=True,
                        )
                    # fold the V bias in as a rank-1 update (bvrow^T @ ones)
                    # so no DVE/ACT bias-add is needed before the transpose
                    nc.tensor.matmul(
                        ps_t[:, jt * M:(jt + 1) * M],
                        bvrow[:, jt * 128:(jt + 1) * 128],
                        ones1[:, :M],
                        start=False, stop=True,
                        skip_group_check=True,
                    )
                # psum -> fp16 on ACT (pool can't touch PSUM; DVE is busy
                # with projection bias-adds around this point)
                nc.scalar.activation(
                    vmT[:, :, :M],
                    ps_t[:, :NJT * M].rearrange("p (j m) -> p j m", m=M),
                    mybir.ActivationFunctionType.Copy,
                )
                vmB = cpool.tile([128, NJT, 128], F16, tag="vmB", name="vmB")
                nc.sync.dma_start_transpose(
                    vmB[:], vmT[:].rearrange("p j m -> p (j m)")
                )
                # head-major relayout + partition-base replication straight
                # from the transposed buffer (no DVE hop)
                for rb in range(4):
                    nc.sync.dma_start(
                        vm[32 * rb:32 * rb + M, :, :HD].rearrange(
                            "m (j u) f -> m j u f", u=2
                        ),
                        vmB[:M, :, :].rearrange("m j (u f) -> m j u f", u=2),
                    )
                memp["vm"] = vm

            # ---- per-chunk phases as generators; the main loop weaves
            # chunk ci's attention with chunk ci+1's projections so the PE
            # always has projection matmuls to stream while Act runs Exps.
            # The handoff is split Q/K vs V: score fills only need Q/K, so
            # they start a V-phase early and V matmuls fill attention tails.
            state_qk = {}
            state_v = {}
            qk_pairs = {}   # (ci, jp) -> (q pair tile, k pair tile)

            def proj_steps(ci):
                xT = x_tiles.pop(ci)
                qT, kT = [], []
                for which, wfn, bias, lst, tg in (
                    (0, wq, bqv, qT, "qT"),
                    (1, wk, bkv, kT, "kT"),
                ):
                    # pp-buf pairs of jt-pair fills, split at the 1MB
                    # weight-DMA boundary (d 0-3 | 4-7) so chunk 0's
                    # matmuls track weight arrival
                    for hw in range(2):
                        pss = []
                        for j2 in range(2):
                            jp = 2 * hw + j2
                            ps_q = pp_pool.tile(
                                [128, 512], F32, tag="pp", name="ps_q"
                            )
                            for u in range(2):
                                jt = 2 * jp + u
                                for d in range(NJT // 2):
                                    nc.tensor.matmul(
                                        ps_q[:, u * 256:(u + 1) * 256],
                                        wfn(d)[:, jt * 128:(jt + 1) * 128],
                                        xT[d][:],
                                        start=(u == 0 and d == 0),
                                        stop=False,
                                    )
                            pss.append(ps_q)
                        for j2 in range(2):
                            jp = 2 * hw + j2
                            ps_q = pss[j2]
                            for u in range(2):
                                jt = 2 * jp + u
                                for d in range(NJT // 2, NJT):
                                    nc.tensor.matmul(
                                        ps_q[:, u * 256:(u + 1) * 256],
                                        wfn(d)[:, jt * 128:(jt + 1) * 128],
                                        xT[d][:],
                                        start=False,
                                        stop=(u == 1 and d == NJT - 1),
                                    )
                            pair_t = qkpool.tile(
                                [128, 2, W], F16, tag=tg, name=f"pair_{tg}"
                            )
                            nc.vector.tensor_tensor(
                                pair_t[:],
                                ps_q[:].rearrange("p (u t) -> p u t", u=2),
                                bias[:, 2 * jp:2 * jp + 2][:, :, None]
                                .to_broadcast((128, 2, W)),
                                mybir.AluOpType.add,
                            )
                            lst.append(pair_t)
                            if which == 1:
                                # publish the (q, k) head-quad as soon as its
                                # K pair lands so the next chunk's fills (and
                                # their Exps) start mid-projection instead of
                                # after the whole K section
                                qk_pairs[(ci, jp)] = (qT[jp], pair_t)
                            yield
                    if ci == 0 and which == 1:
                        emit_ktm()
                        yield
                state_qk[ci] = (
                    [qT[jt // 2][:, jt % 2, :] for jt in range(NJT)],
                    [kT[jt // 2][:, jt % 2, :] for jt in range(NJT)],
                )

                # V projection (token-major fp16, heads + ones col)
                v_sb = vpool.tile([128, 2, H, VW], F16, tag="v_sb")
                nc.gpsimd.memset(v_sb[:, :, :, HD:HD + 1], 1.0)
                for tt in range(2):
                    pss = []
                    for half in range(2):
                        ps_v = pp_pool.tile([128, 512], F32, tag="pp")
                        for d in range(NJT // 2):
                            nc.tensor.matmul(
                                ps_v[:], xT[d][:, tt * 128:(tt + 1) * 128],
                                wv(d)[:, half * 512:(half + 1) * 512],
                                start=(d == 0), stop=False,
                            )
                        pss.append(ps_v)
                    for half in range(2):
                        ps_v = pss[half]
                        for d in range(NJT // 2, NJT):
                            nc.tensor.matmul(
                                ps_v[:], xT[d][:, tt * 128:(tt + 1) * 128],
                                wv(d)[:, half * 512:(half + 1) * 512],
                                start=False, stop=(d == NJT - 1),
                            )
                        nc.vector.tensor_tensor(
                            v_sb[:, tt, half * 8:(half + 1) * 8, :HD],
                            ps_v[:].rearrange("p (h f) -> p h f", h=8),
                            bvb[:, half * 512:(half + 1) * 512].rearrange(
                                "p (h f) -> p h f", h=8
                            ),
                            mybir.AluOpType.add,
                        )
                        yield
                    if ci == 0 and tt == 0:
                        emit_vm()
                state_v[ci] = v_sb

            def attn_steps(ci):
                last = ci == CPC - 1

                # mem scores: all 16 heads in one 2-bank psum, one Exp.
                # Head pair jt lands at 32-aligned partition bases via the
                # block-diagonal stationary and tile_position cols.
                def emit_ms():
                    qT, kT = state_qk.pop(ci)
                    kTm3 = memp["kTm3"]
                    ps_m = ps_pool.tile([128, 1024], F32, tag="ps", name="ps_ms")
                    for jt in range(NJT):
                        c0 = 64 * (jt % 2)
                        g = jt // 2
                        nc.tensor.matmul(
                            ps_m[c0:c0 + 64, g * 256:(g + 1) * 256],
                            kTm3[:, jt, :],
                            qT[jt][:],
                            start=True, stop=True,
                            tile_position=(0, c0),
                            skip_group_check=True,
                        )
                    em = empool.tile([128, 4, 256], F16, tag="em", name="em")
                    nc.scalar.activation(
                        em[:], ps_m[:].rearrange("p (g t) -> p g t", g=4),
                        mybir.ActivationFunctionType.Exp,
                        bias=mskv[:, ci * 3 + 2: ci * 3 + 3],
                    )
                    return em

                out_sb = opool.tile([128, 2, D], F16, tag="out_sb", name="out_sb")
                eloc = {}   # (hpq, yt) -> [128, 1024] fp16

                def fill(hpq, yt):
                    # local scoresT for 4 heads (one quad, one key half);
                    # both 64-row ab halves quadrant-packed; one Exp
                    qp, kp = qk_pairs[(ci, hpq)]
                    ps_s = ps_pool.tile([128, 1024], F32, tag="ps", name="ps_s")
                    for ab in range(2):
                        p0 = 64 * ab
                        for u in range(2):
                            nc.tensor.matmul(
                                ps_s[:, ab * 512 + u * 256:
                                     ab * 512 + (u + 1) * 256],
                                kp[p0:p0 + 64, u, yt * 128:(yt + 1) * 128],
                                qp[p0:p0 + 64, u, :],
                                start=(u == 0), stop=(u == 1),
                                tile_position=(p0, 0),
                                skip_group_check=True,
                            )
                    e_t = epool.tile([128, 1024], F16, tag="exps")
                    nc.scalar.activation(
                        e_t[:], ps_s[:],
                        mybir.ActivationFunctionType.Exp,
                        bias=mskv[:, ci * 3 + yt: ci * 3 + yt + 1],
                    )
                    eloc[(hpq, yt)] = e_t

                def unit_mm(hp):
                    # PV for head pair hp: one psum bank, 4 blocks (h, xb)
                    # of 65 (64 hd + denom)
                    ps_o = po_pool.tile([128, 4 * VW], F32, tag="po")
                    for ab in range(2):
                        h = 2 * hp + ab
                        e0 = eloc[(hp // 2, 0)]
                        e1 = eloc[(hp // 2, 1)]
                        base, g = _mem_slot(h)
                        for xb in range(2):
                            o = (ab * 2 + xb) * VW
                            xs = (h % 2) * 512 + (hp % 2) * 256 + xb * 128
                            nc.tensor.matmul(
                                ps_o[:, o:o + VW],
                                e0[:, xs:xs + 128], v_sb[:, 0, h, :],
                                start=True, stop=False,
                            )
                            nc.tensor.matmul(
                                ps_o[:, o:o + VW],
                                e1[:, xs:xs + 128], v_sb[:, 1, h, :],
                                start=False, stop=False,
                            )
                            nc.tensor.matmul(
                                ps_o[:, o:o + VW],
                                em[base:base + M, g, xb * 128:(xb + 1) * 128],
                                vm[base:base + M, h, :],
                                start=False, stop=True,
                                tile_position=(base, 0),
                            )
                    return ps_o

                def unit_recip(hp, ps_o):
                    rec = rpool.tile([128, 4], F32, tag="rec", name="rec")
                    nc.vector.reciprocal(
                        rec[:].rearrange("p (k o) -> p k o", o=1),
                        ps_o[:].rearrange("p (k w) -> p k w", w=VW)[
                            :, :, HD:HD + 1
                        ],
                    )
                    return rec

                def unit_mult(hp, ps_o, rec):
                    nc.vector.tensor_tensor(
                        out_sb[:, :, 2 * hp * HD:(2 * hp + 2) * HD].rearrange(
                            "p x (a f) -> p a x f", a=2
                        ),
                        ps_o[:].rearrange("p (a x w) -> p a x w", a=2, x=2)[
                            :, :, :, :HD
                        ],
                        rec[:].rearrange("p (a x) -> p a x", a=2)[
                            :, :, :, None
                        ].to_broadcast((128, 2, 2, HD)),
                        mybir.AluOpType.mult,
                    )

                def unit(hp):
                    ps_o = unit_mm(hp)
                    unit_mult(hp, ps_o, unit_recip(hp, ps_o))

                def out_dma(qtr):
                    nc.sync.dma_start(
                        out_d.rearrange("(x p) c -> p x c", p=128)[
                            :, 2 * ci:2 * ci + 2, qtr * 256:(qtr + 1) * 256
                        ],
                        out_sb[:, :, qtr * 256:(qtr + 1) * 256],
                    )

                # fills gate on their own (q, k) head-pair so they start
                # mid-K-projection; Exps spread forward on ACT. The last
                # chunk front-loads ALL fills (spaced a round apart for psum
                # recycling) so its Exps finish during proj(last)'s V section
                # and the closing PV units never wait on ACT.
                while ci not in state_qk:
                    yield
                fill(0, 0); yield
                fill(0, 1); yield
                em = emit_ms(); yield
                fill(1, 0); yield
                fill(1, 1); yield
                while ci not in state_v:
                    yield
                v_sb = state_v.pop(ci)
                vm = memp["vm"]
                unit(0); yield
                unit(1); out_dma(0); yield
                fill(2, 0); yield
                fill(2, 1); yield
                if last:
                    fill(3, 0); yield
                    fill(3, 1); yield
                unit(2); yield
                unit(3); out_dma(1); yield
                if not last:
                    fill(3, 0); yield
                    fill(3, 1); yield
                unit(4); yield
                unit(5); out_dma(2); yield
                # last pair: defer unit 6's normalize past unit 7's matmuls
                # and reciprocal so only recip7+mult7 trail the PE stream
                ps6 = unit_mm(6)
                rec6 = unit_recip(6, ps6); yield
                ps7 = unit_mm(7)
                unit_mult(6, ps6, rec6)
                rec7 = unit_recip(7, ps7)
                unit_mult(7, ps7, rec7)
                out_dma(3)

            def drain(*gens):
                gens = [g for g in gens if g is not None]
                while gens:
                    nxt = []
                    for g in gens:
                        try:
                            next(g)
                            nxt.append(g)
                        except StopIteration:
                            pass
                    gens = nxt

            # 3-way weave: chunk ci's attention runs with chunk ci+1's
            # projections, and attn(ci+1) joins early (it self-waits on
            # its state) so the attention tail always has matmul filler
            attns_g = [attn_steps(ci) for ci in range(CPC)]
            p0 = proj_steps(0)
            gens0 = [p0, attns_g[0]]
            must0 = {id(p0)}
            while must0:
                for g in list(gens0):
                    try:
                        next(g)
                    except StopIteration:
                        gens0.remove(g)
                        must0.discard(id(g))
            for ci in range(CPC):
                gens = [attns_g[ci]]
                must = {id(attns_g[ci])}
                if ci + 1 < CPC:
                    pj = proj_steps(ci + 1)
                    # proj(ci+1) and attn(ci+1) step twice per round so the
                    # next chunk's projections and fills/Exps run far enough
                    # ahead that the closing PV units never wait on an Exp
                    # (matters most for the last chunk, which runs with no
                    # projection filler)
                    gens = [pj, attns_g[ci],
                            attns_g[ci + 1], attns_g[ci + 1]]
                    must.add(id(pj))
                while must:
                    for g in list(gens):
                        try:
                            next(g)
                        except StopIteration:
                            if g in gens:
                                gens.remove(g)
                            must.discard(id(g))

    nc.compile()
    return nc


_NC_CACHE = None


def kernel(hidden_states, attention_mask, self_memory, Wq, bq, Wk, bk, Wv, bv):
    global _NC_CACHE, LAST_RESULTS
    hidden_states = np.asarray(np.asarray(hidden_states), np.float32)
    attention_mask = np.asarray(np.asarray(attention_mask), np.float32)
    self_memory = np.asarray(np.asarray(self_memory), np.float32)
    wqT = np.ascontiguousarray(
        (np.asarray(Wq, np.float32).T * 0.125).astype(np.float16)
    )
    wkT = np.ascontiguousarray(np.asarray(Wk, np.float32).T.astype(np.float16))
    wvT = np.ascontiguousarray(np.asarray(Wv, np.float32).T.astype(np.float16))
    bqv = np.ascontiguousarray(
        np.asarray(bq, np.float32).reshape(NJT, 128).T * 0.125
    )
    bkv = np.ascontiguousarray(np.asarray(bk, np.float32).reshape(NJT, 128).T)
    bvbr = np.ascontiguousarray(np.asarray(bv, np.float32).reshape(1, D))
    bvbr16 = bvbr.astype(np.float16)

    # additive mask along the key axis, per (b, c): [yt0 | yt1 | memory].
    # Clamped to -11: softmax is shift-invariant, so for a fully-masked
    # chunk exp(s - 11) still normalizes to softmax(s) (matching the
    # reference) instead of underflowing fp16 to 0/0; for partial masks
    # the e^-11 leakage per masked key is ~2e-5 of a valid key.
    am = np.maximum(attention_mask.reshape(B, C, W), -11.0)
    chunk_has_valid = (attention_mask.reshape(B, C, W) == 0.0).sum(axis=2) > 0
    # the reference broadcasts mem_mask[:, None, None, :] over the
    # memory-TOKEN axis (M == C): mem token j is masked for every chunk
    # of batch b iff chunk j of batch b is fully masked. As a per-em-
    # partition bias vector: token j sits at partitions {32*rb + j}.
    mem_mask = np.where(chunk_has_valid, 0.0, -11.0).astype(np.float32)  # [B, C=M]
    memv = np.zeros((B, 128), np.float32)
    for rb in range(4):
        memv[:, 32 * rb:32 * rb + M] = mem_mask

    if _NC_CACHE is None:
        _NC_CACHE = _build_kernel()
    nc = _NC_CACHE

    x16 = hidden_states.astype(np.float16)
    mem16 = self_memory.astype(np.float16)

    in_maps = []
    for core in range(N_CORES):
        b = core // (N_CORES // B)
        c0 = (core % (N_CORES // B)) * CPC
        mvT = np.zeros((128, CPC * 3), np.float32)
        for ci in range(CPC):
            mvT[:, ci * 3 + 0] = am[b, c0 + ci, 0:128]
            mvT[:, ci * 3 + 1] = am[b, c0 + ci, 128:256]
            mvT[:, ci * 3 + 2] = memv[b]
        # feature-major pre-transposed x: [128, NJT, TPC]
        xT = np.ascontiguousarray(
            x16[b, c0 * W:(c0 + CPC) * W, :]
            .T.reshape(NJT, 128, TPC).transpose(1, 0, 2)
        ).reshape(128, NJT * TPC)
        memT = np.ascontiguousarray(
            mem16[b].T.reshape(NJT, 128, M).transpose(1, 0, 2)
        ).reshape(128, NJT * M)
        in_maps.append(
            {
                "xT": xT,
                "memT": memT,
                "wqT": wqT,
                "wkT": wkT,
                "wvT": wvT,
                "bqv": bqv,
                "bkv": bkv,
                "bvbr": bvbr,
                "bvbr16": bvbr16,
                "maskvT": mvT,
            }
        )

    res = run_bass_kernel_spmd(nc, in_maps, list(range(N_CORES)), trace=TRACE)
    LAST_RESULTS = res

    out = np.empty((B, S, D), np.float32)
    for core in range(N_CORES):
        b = core // (N_CORES // B)
        c0 = (core % (N_CORES // B)) * CPC
        out[b, c0 * W:(c0 + CPC) * W, :] = res.results[core]["out"].astype(
            np.float32
        )
    return out



# revision 60
# speedup vs baseline: 1.0032x; 1.0032x over previous
"""Bass/Trainium2 kernel for chunked local attention with memory tokens
(BertSelfAttention variant). Self-contained: hardcodes all shapes.

Sharding: 8 cores, each handles 4 of the 32 (batch, chunk) pairs.
  core i -> b = i // 4, chunks 4*(i % 4) .. 4*(i % 4) + 3
No collectives; weights replicated per core; host scatters/gathers.

Per-core device computation (PE operands fp16, accumulation fp32):
  - xT [128, 8, 1024] arrives pre-transposed from host (feature-major),
    sliced per chunk; no device transposes
  - qT[jt] = x@(Wq/8) + bq/8 (feature-major)   kT[jt] = x@Wk + bk
  - v[t, yt, h, 65] token-major with a ones column at 64 that makes the
    PV matmul emit softmax denominators
  - memory-token K is materialized block-diagonally (kTm3 [128, jt, 64]:
    rows 0:64 x cols 0:16 = even head, rows 64:128 x cols 32:48 = odd) so
    one 128-contraction matmul scores 2 heads' mem keys at 32-aligned
    psum partitions; all 16 heads' mem scores share one 2-bank psum and
    a single Exp
  - local scoresT per (head-quad, yt) fill a 2-bank [128, 1024] psum
    (both 64-row ab halves, quadrant-packed) -> one Exp per fill
  - probs = Exp(scoresT + mask[y]) in fp16; additive key mask rides the
    ACT bias operand (per-partition scalar)
  - out_unnorm[x, 4*(hd|den)] per head-pair in one psum bank; strided
    reciprocal + one broadcast multiply normalize and emit fp16 out
Output fp16 on device, upcast to fp32 on host.

Scheduling: generator-based software pipelining weaves chunk ci's
attention with chunk ci+1's projections (attn(ci+1) steps twice per
round so its fills/Exps run far ahead) so the PE streams matmuls while
ACT runs the Exps; tiny warm matmuls keep the PE p-state ramp clock
alive across startup DMA waits.

DMA orchestration (the cost model charges ~625ns serialized HWDGE issue
+ serial transfers on one DMA-engine set + 900ns sem per DMA):
  - Q/K weights stream on the sync queue as 256KB granules in the exact
    order chunk-0's psum fills consume them, with x(0) halves
    interleaved at first use -> first projection matmul at ~4.4us
    (DMA-pipe floor) instead of ~7.8us
  - small constants (biases, mask, mem tokens) go through the Pool
    engine's SWDGE path, off the HWDGE critical chain; the V bias
    broadcast [128, D] is a DMA (no PE/DVE work)
  - mem-V vm: bias folded into the psum as a rank-1 (bvrow^T @ ones)
    matmul, ACT copies psum->fp16 (Pool cannot touch PSUM), and the
    head-major relayout + 4 partition-base replicas come straight off
    the transposed buffer as DMAs (no DVE hop)
  - the last chunk defers unit 6's normalize past unit 7's matmuls so
    only recip7+mult7+one 256KB DMA trail the final PE instruction
"""

import sys

sys.path.insert(0, "/opt/trn_rl_repo")

import numpy as np

import concourse.tile as tile
from concourse import bacc, mybir
from concourse.bass_utils import run_bass_kernel_spmd

F32 = mybir.dt.float32
F16 = mybir.dt.float16

B, S, D = 2, 4096, 1024
H, HD = 16, 64
W = 256            # attention window (chunk length)
C = S // W         # 16 chunks
M = 16             # memory tokens
N_CORES = 8
CPC = C * B // N_CORES  # 4 chunks per core
TPC = CPC * W           # 1024 chunk tokens per core
NJT = D // 128          # 8 feature tiles
VW = HD + 1             # v width (64 hd + ones col)

TRACE = False
LAST_RESULTS = None


def _mem_slot(h):
    """(partition base, free block) of head h's mem scores / probs."""
    jt, u = h // 2, h % 2
    return 64 * (jt % 2) + 32 * u, jt // 2


def _build_kernel():
    nc = bacc.Bacc(None, target_bir_lowering=False)

    xT_d = nc.declare_dram_parameter("xT", [128, NJT * TPC], F16, isOutput=False)
    memT_d = nc.declare_dram_parameter("memT", [128, NJT * M], F16, isOutput=False)
    wqT_d = nc.declare_dram_parameter("wqT", [D, D], F16, isOutput=False)
    wkT_d = nc.declare_dram_parameter("wkT", [D, D], F16, isOutput=False)
    wvT_d = nc.declare_dram_parameter("wvT", [D, D], F16, isOutput=False)
    bq_d = nc.declare_dram_parameter("bqv", [128, NJT], F32, isOutput=False)
    bk_d = nc.declare_dram_parameter("bkv", [128, NJT], F32, isOutput=False)
    bvr_d = nc.declare_dram_parameter("bvbr", [1, D], F32, isOutput=False)
    bvr16_d = nc.declare_dram_parameter("bvbr16", [1, D], F16, isOutput=False)
    msk_d = nc.declare_dram_parameter("maskvT", [128, CPC * 3], F32, isOutput=False)
    out_d = nc.declare_dram_parameter("out", [TPC, D], F16, isOutput=True)

    with tile.TileContext(nc) as tc:
        with (
            tc.tile_pool(name="const", bufs=1) as cpool,
            tc.tile_pool(name="wpool", bufs=1) as wpool,
            tc.tile_pool(name="xtpool", bufs=4) as xtpool,
            tc.tile_pool(name="qkpool", bufs=16) as qkpool,
            tc.tile_pool(name="vpool", bufs=4) as vpool,
            tc.tile_pool(name="epool", bufs=16) as epool,
            tc.tile_pool(name="empool", bufs=3) as empool,
            tc.tile_pool(name="opool", bufs=4) as opool,
            tc.tile_pool(name="rpool", bufs=4) as rpool,
            tc.tile_pool(name="pp", bufs=2, space="PSUM") as pp_pool,
            tc.tile_pool(name="ps", bufs=2, space="PSUM") as ps_pool,
            tc.tile_pool(name="po", bufs=2, space="PSUM") as po_pool,
        ):
            x_tiles = {}

            def load_x(ci, eng, split=False):
                x_t = xtpool.tile([128, NJT, W], F16, tag="xT", name="xT")
                xs = xT_d.rearrange("p (o t) -> p o t", t=TPC)[
                    :, :, ci * W:(ci + 1) * W
                ]
                if split:
                    eng.dma_start(x_t[:, 0:4, :], xs[:, 0:4, :])
                    eng.dma_start(x_t[:, 4:8, :], xs[:, 4:8, :])
                else:
                    eng.dma_start(x_t[:], xs)
                x_tiles[ci] = [x_t[:, d, :] for d in range(NJT)]

            w_all = wpool.tile([128, 3 * NJT, D], F16, tag="w_all")

            def load_w_piece(wi, wd, o0, og, c0, cw, eng):
                eng.dma_start(
                    w_all[:, wi * NJT + o0: wi * NJT + o0 + og, c0:c0 + cw],
                    wd.rearrange("(o p) c -> p o c", p=128)[
                        :, o0:o0 + og, c0:c0 + cw
                    ],
                )

            def wq(d):
                return w_all[:, d, :]

            def wk(d):
                return w_all[:, NJT + d, :]

            def wv(d):
                return w_all[:, 2 * NJT + d, :]

            # ---- pool-queue constants (SWDGE path, bypasses the HWDGE
            # serialization on the sync queue): ones first so the PE warm
            # matmul fires ~250ns in, then small constants in consumption
            # order, then the V weights (needed only mid-chunk-0).
            ones1 = cpool.tile([1, 128], F16, tag="ones1")
            nc.gpsimd.memset(ones1[:], 1.0)
            bqv = cpool.tile([128, NJT], F32, tag="bqv")
            nc.gpsimd.dma_start(bqv[:], bq_d[:])
            bkv = cpool.tile([128, NJT], F32, tag="bkv")
            nc.gpsimd.dma_start(bkv[:], bk_d[:])
            mskv = cpool.tile([128, CPC * 3], F32, tag="mskv")
            nc.gpsimd.dma_start(mskv[:], msk_d[:])
            xTm = cpool.tile([128, NJT, M], F16, tag="xTm")
            nc.gpsimd.dma_start(xTm[:], memT_d.rearrange("p (o m) -> p o m", m=M))
            bvrow = cpool.tile([1, D], F16, tag="bvrow")
            nc.gpsimd.dma_start(bvrow[:], bvr16_d[:])
            bvb = cpool.tile([128, D], F32, tag="bvb")

            # ---- sync-queue streams: 128KB weight granules in the exact
            # order chunk-0's psum fills consume them (jp-pair cols x d-half
            # rows), with x(0) halves interleaved at their first use.
            x_t0 = xtpool.tile([128, NJT, W], F16, tag="xT", name="xT")
            xs0 = xT_d.rearrange("p (o t) -> p o t", t=TPC)[:, :, 0:W]
            nc.sync.dma_start(x_t0[:, 0:4, :], xs0[:, 0:4, :])
            x_tiles[0] = [x_t0[:, d, :] for d in range(NJT)]
            # 256KB granules in chunk-0 consumption order (jp-pair cols x
            # d-half rows)
            GRAN = ((0, 0), (0, 256), (4, 0), (4, 256),
                    (0, 512), (0, 768), (4, 512), (4, 768))
            for o0, c0 in GRAN[:2]:
                load_w_piece(0, wqT_d, o0, 4, c0, 256, nc.sync)
            nc.sync.dma_start(x_t0[:, 4:8, :], xs0[:, 4:8, :])
            for o0, c0 in GRAN[2:]:
                load_w_piece(0, wqT_d, o0, 4, c0, 256, nc.sync)
            for o0, c0 in GRAN:
                load_w_piece(1, wkT_d, o0, 4, c0, 256, nc.sync)
            load_w_piece(2, wvT_d, 0, 4, 0, D, nc.sync)
            load_w_piece(2, wvT_d, 4, 4, 0, D, nc.sync)
            nc.sync.dma_start(bvb[:], bvr_d[:].to_broadcast((128, D)))
            load_x(1, nc.sync)
            load_x(2, nc.sync)
            load_x(3, nc.sync)

            # tiny dep-free matmuls: start the PE p-state ramp clock early
            # (ones lands ~250ns via the pool memset) and refresh it off the
            # small pool DMAs so the idle gaps before the first projection
            # fills stay under the ~3us p-state reset threshold
            ps_warm = pp_pool.tile([128, 512], F32, tag="pp")
            nc.tensor.matmul(
                ps_warm[:16, :16], ones1[:, :16], ones1[:, :16],
                start=True, stop=True,
            )
            nc.tensor.matmul(
                ps_warm[:4, 16:20], bqv[:16, :4], bqv[:16, :4],
                start=True, stop=True,
            )
            x0d0 = x_tiles[0][0]
            nc.tensor.matmul(
                ps_warm[:16, 32:48], x0d0[:16, :16], x0d0[:16, :16],
                start=True, stop=True,
            )

            # ---- memory tokens (emitted inside chunk 0's flow) ----
            memp = {}

            def emit_ktm():
                # block-diagonal mem-K: [128, jt, 64]; even head rows 0:64 ->
                # cols 0:16, odd head rows 64:128 -> cols 32:48, rest zero
                kTm3 = cpool.tile([128, NJT, 64], F16, tag="kTm3", name="kTm3")
                nc.gpsimd.memset(kTm3[:], 0.0)
                ps_k = pp_pool.tile([128, 512], F32, tag="pp", name="ps_ktm")
                for jt in range(NJT):
                    for d in range(NJT):
                        nc.tensor.matmul(
                            ps_k[:, jt * M:(jt + 1) * M],
                            wk(d)[:, jt * 128:(jt + 1) * 128],
                            xTm[:, d, :],
                            start=(d == 0), stop=(d == NJT - 1),
                            skip_group_check=True,
                        )
                for u in range(2):
                    nc.vector.tensor_tensor(
                        kTm3[64 * u:64 * (u + 1), :, 32 * u:32 * u + 16],
                        ps_k[64 * u:64 * (u + 1), :NJT * M].rearrange(
                            "p (j m) -> p j m", m=M
                        ),
                        bkv[64 * u:64 * (u + 1), :][:, :, None].to_broadcast(
                            (64, NJT, M)
                        ),
                        mybir.AluOpType.add,
                    )
                memp["kTm3"] = kTm3

            def emit_vm():
                # mem-V computed feature-major in one cheap psum fill, then
                # token-major via an xbar DMA-transpose of the m-padded
                # [128, jt, 128] layout (dst[p, jt, f] = src[f, jt*128+p]);
                # replicated at partition bases 0/32/64/96 for the
                # 32-aligned mem-PV stationaries
                vm = cpool.tile([128, M, VW], F16, tag="vm", name="vm")
                for rb in range(4):
                    nc.gpsimd.memset(vm[32 * rb:32 * rb + M, :, HD:HD + 1], 1.0)
                vmT = cpool.tile([128, NJT, 128], F16, tag="vmT", name="vmT")
                nc.gpsimd.memset(vmT[:], 0.0)
                ps_t = pp_pool.tile([128, 512], F32, tag="pp", name="ps_vm")
                for jt in range(NJT):
                    for d in range(NJT):
                        nc.tensor.matmul(
                            ps_t[:, jt * M:(jt + 1) * M],
                            wv(d)[:, jt * 128:(jt + 1) * 128],
                            xTm[:, d, :],
                            start=(d == 0), stop=False,
                            skip_group_check=True,
                        )
                    # fold the V bias in as a rank-1 update (bvrow^T @ ones)
                    # so no DVE/ACT bias-add is needed before the transpose
                    nc.tensor.matmul(
                        ps_t[:, jt * M:(jt + 1) * M],
                        bvrow[:, jt * 128:(jt + 1) * 128],
                        ones1[:, :M],
                        start=False, stop=True,
                        skip_group_check=True,
                    )
                # psum -> fp16 on ACT (pool can't touch PSUM; DVE is busy
                # with projection bias-adds around this point)
                nc.scalar.activation(
                    vmT[:, :, :M],
                    ps_t[:, :NJT * M].rearrange("p (j m) -> p j m", m=M),
                    mybir.ActivationFunctionType.Copy,
                )
                vmB = cpool.tile([128, NJT, 128], F16, tag="vmB", name="vmB")
                nc.sync.dma_start_transpose(
                    vmB[:], vmT[:].rearrange("p j m -> p (j m)")
                )
                # head-major relayout + partition-base replication straight
                # from the transposed buffer (no DVE hop)
                for rb in range(4):
                    nc.sync.dma_start(
                        vm[32 * rb:32 * rb + M, :, :HD].rearrange(
                            "m (j u) f -> m j u f", u=2
                        ),
                        vmB[:M, :, :].rearrange("m j (u f) -> m j u f", u=2),
                    )
                memp["vm"] = vm

            # ---- per-chunk phases as generators; the main loop weaves
            # chunk ci's attention with chunk ci+1's projections so the PE
            # always has projection matmuls to stream while Act runs Exps.
            # The handoff is split Q/K vs V: score fills only need Q/K, so
            # they start a V-phase early and V matmuls fill attention tails.
            state_qk = {}
            state_v = {}
            state_v_lo = {}  # heads 0:8 complete (units 0-3 can start)
            qk_pairs = {}   # (ci, jp) -> (q pair tile, k pair tile)

            def proj_steps(ci):
                xT = x_tiles.pop(ci)
                qT, kT = [], []
                for which, wfn, bias, lst, tg in (
                    (0, wq, bqv, qT, "qT"),
                    (1, wk, bkv, kT, "kT"),
                ):
                    # pp-buf pairs of jt-pair fills, split at the 1MB
                    # weight-DMA boundary (d 0-3 | 4-7) so chunk 0's
                    # matmuls track weight arrival
                    for hw in range(2):
                        pss = []
                        for j2 in range(2):
                            jp = 2 * hw + j2
                            ps_q = pp_pool.tile(
                                [128, 512], F32, tag="pp", name="ps_q"
                            )
                            for u in range(2):
                                jt = 2 * jp + u
                                for d in range(NJT // 2):
                                    nc.tensor.matmul(
                                        ps_q[:, u * 256:(u + 1) * 256],
                                        wfn(d)[:, jt * 128:(jt + 1) * 128],
                                        xT[d][:],
                                        start=(u == 0 and d == 0),
                                        stop=False,
                                    )
                            pss.append(ps_q)
                        for j2 in range(2):
                            jp = 2 * hw + j2
                            ps_q = pss[j2]
                            for u in range(2):
                                jt = 2 * jp + u
                                for d in range(NJT // 2, NJT):
                                    nc.tensor.matmul(
                                        ps_q[:, u * 256:(u + 1) * 256],
                                        wfn(d)[:, jt * 128:(jt + 1) * 128],
                                        xT[d][:],
                                        start=False,
                                        stop=(u == 1 and d == NJT - 1),
                                    )
                            pair_t = qkpool.tile(
                                [128, 2, W], F16, tag=tg, name=f"pair_{tg}"
                            )
                            nc.vector.tensor_tensor(
                                pair_t[:],
                                ps_q[:].rearrange("p (u t) -> p u t", u=2),
                                bias[:, 2 * jp:2 * jp + 2][:, :, None]
                                .to_broadcast((128, 2, W)),
                                mybir.AluOpType.add,
                            )
                            lst.append(pair_t)
                            if which == 1:
                                # publish the (q, k) head-quad as soon as its
                                # K pair lands so the next chunk's fills (and
                                # their Exps) start mid-projection instead of
                                # after the whole K section
                                qk_pairs[(ci, jp)] = (qT[jp], pair_t)
                            yield
                    if ci == 0 and which == 1:
                        emit_ktm()
                        yield
                state_qk[ci] = (
                    [qT[jt // 2][:, jt % 2, :] for jt in range(NJT)],
                    [kT[jt // 2][:, jt % 2, :] for jt in range(NJT)],
                )

                # V projection (token-major fp16, heads + ones col)
                v_sb = vpool.tile([128, 2, H, VW], F16, tag="v_sb")
                nc.gpsimd.memset(v_sb[:, :, :, HD:HD + 1], 1.0)
                for tt in range(2):
                    pss = []
                    for half in range(2):
                        ps_v = pp_pool.tile([128, 512], F32, tag="pp")
                        for d in range(NJT // 2):
                            nc.tensor.matmul(
                                ps_v[:], xT[d][:, tt * 128:(tt + 1) * 128],
                                wv(d)[:, half * 512:(half + 1) * 512],
                                start=(d == 0), stop=False,
                            )
                        pss.append(ps_v)
                        if ci == CPC - 1:
                            # finer V yields for the last projection so the
                            # last chunk's fills slot in at ~850ns cadence
                            # and its Exp chain never starves
                            yield
                    for half in range(2):
                        ps_v = pss[half]
                        for d in range(NJT // 2, NJT):
                            nc.tensor.matmul(
                                ps_v[:], xT[d][:, tt * 128:(tt + 1) * 128],
                                wv(d)[:, half * 512:(half + 1) * 512],
                                start=False, stop=(d == NJT - 1),
                            )
                        nc.vector.tensor_tensor(
                            v_sb[:, tt, half * 8:(half + 1) * 8, :HD],
                            ps_v[:].rearrange("p (h f) -> p h f", h=8),
                            bvb[:, half * 512:(half + 1) * 512].rearrange(
                                "p (h f) -> p h f", h=8
                            ),
                            mybir.AluOpType.add,
                        )
                        if tt == 1 and half == 0:
                            # heads 0:8 complete for both token halves:
                            # units 0-3 can start under the V tail
                            state_v_lo[ci] = v_sb
                        yield
                    if ci == 0 and tt == 0:
                        emit_vm()
                state_v[ci] = v_sb

            def attn_steps(ci):
                last = ci == CPC - 1

                # mem scores: all 16 heads in one 2-bank psum, one Exp.
                # Head pair jt lands at 32-aligned partition bases via the
                # block-diagonal stationary and tile_position cols.
                def emit_ms():
                    qT, kT = state_qk.pop(ci)
                    kTm3 = memp["kTm3"]
                    ps_m = ps_pool.tile([128, 1024], F32, tag="ps", name="ps_ms")
                    for jt in range(NJT):
                        c0 = 64 * (jt % 2)
                        g = jt // 2
                        nc.tensor.matmul(
                            ps_m[c0:c0 + 64, g * 256:(g + 1) * 256],
                            kTm3[:, jt, :],
                            qT[jt][:],
                            start=True, stop=True,
                            tile_position=(0, c0),
                            skip_group_check=True,
                        )
                    em = empool.tile([128, 4, 256], F16, tag="em", name="em")
                    nc.scalar.activation(
                        em[:], ps_m[:].rearrange("p (g t) -> p g t", g=4),
                        mybir.ActivationFunctionType.Exp,
                        bias=mskv[:, ci * 3 + 2: ci * 3 + 3],
                    )
                    return em

                out_sb = opool.tile([128, 2, D], F16, tag="out_sb", name="out_sb")
                eloc = {}   # (hpq, yt) -> [128, 1024] fp16

                def fill(hpq, yt):
                    # local scoresT for 4 heads (one quad, one key half);
                    # both 64-row ab halves quadrant-packed; one Exp
                    qp, kp = qk_pairs[(ci, hpq)]
                    ps_s = ps_pool.tile([128, 1024], F32, tag="ps", name="ps_s")
                    for ab in range(2):
                        p0 = 64 * ab
                        for u in range(2):
                            nc.tensor.matmul(
                                ps_s[:, ab * 512 + u * 256:
                                     ab * 512 + (u + 1) * 256],
                                kp[p0:p0 + 64, u, yt * 128:(yt + 1) * 128],
                                qp[p0:p0 + 64, u, :],
                                start=(u == 0), stop=(u == 1),
                                tile_position=(p0, 0),
                                skip_group_check=True,
                            )
                    e_t = epool.tile([128, 1024], F16, tag="exps")
                    nc.scalar.activation(
                        e_t[:], ps_s[:],
                        mybir.ActivationFunctionType.Exp,
                        bias=mskv[:, ci * 3 + yt: ci * 3 + yt + 1],
                    )
                    eloc[(hpq, yt)] = e_t

                def unit_mm(hp):
                    # PV for head pair hp: one psum bank, 4 blocks (h, xb)
                    # of 65 (64 hd + denom)
                    ps_o = po_pool.tile([128, 4 * VW], F32, tag="po")
                    for ab in range(2):
                        h = 2 * hp + ab
                        e0 = eloc[(hp // 2, 0)]
                        e1 = eloc[(hp // 2, 1)]
                        base, g = _mem_slot(h)
                        for xb in range(2):
                            o = (ab * 2 + xb) * VW
                            xs = (h % 2) * 512 + (hp % 2) * 256 + xb * 128
                            nc.tensor.matmul(
                                ps_o[:, o:o + VW],
                                e0[:, xs:xs + 128], v_sb[:, 0, h, :],
                                start=True, stop=False,
                            )
                            nc.tensor.matmul(
                                ps_o[:, o:o + VW],
                                e1[:, xs:xs + 128], v_sb[:, 1, h, :],
                                start=False, stop=False,
                            )
                            nc.tensor.matmul(
                                ps_o[:, o:o + VW],
                                em[base:base + M, g, xb * 128:(xb + 1) * 128],
                                vm[base:base + M, h, :],
                                start=False, stop=True,
                                tile_position=(base, 0),
                            )
                    return ps_o

                def unit_recip(hp, ps_o):
                    rec = rpool.tile([128, 4], F32, tag="rec", name="rec")
                    nc.vector.reciprocal(
                        rec[:].rearrange("p (k o) -> p k o", o=1),
                        ps_o[:].rearrange("p (k w) -> p k w", w=VW)[
                            :, :, HD:HD + 1
                        ],
                    )
                    return rec

                def unit_mult(hp, ps_o, rec):
                    nc.vector.tensor_tensor(
                        out_sb[:, :, 2 * hp * HD:(2 * hp + 2) * HD].rearrange(
                            "p x (a f) -> p a x f", a=2
                        ),
                        ps_o[:].rearrange("p (a x w) -> p a x w", a=2, x=2)[
                            :, :, :, :HD
                        ],
                        rec[:].rearrange("p (a x) -> p a x", a=2)[
                            :, :, :, None
                        ].to_broadcast((128, 2, 2, HD)),
                        mybir.AluOpType.mult,
                    )

                def unit(hp):
                    ps_o = unit_mm(hp)
                    unit_mult(hp, ps_o, unit_recip(hp, ps_o))

                def out_dma(qtr):
                    nc.sync.dma_start(
                        out_d.rearrange("(x p) c -> p x c", p=128)[
                            :, 2 * ci:2 * ci + 2, qtr * 256:(qtr + 1) * 256
                        ],
                        out_sb[:, :, qtr * 256:(qtr + 1) * 256],
                    )

                # fills gate on their own (q, k) head-pair so they start
                # mid-K-projection; Exps spread forward on ACT. The last
                # chunk front-loads ALL fills (spaced a round apart for psum
                # recycling) so its Exps finish during proj(last)'s V section
                # and the closing PV units never wait on ACT.
                while ci not in state_qk:
                    yield
                fill(0, 0); yield
                fill(0, 1); yield
                em = emit_ms(); yield
                fill(1, 0); yield
                fill(1, 1); yield
                if last:
                    # feed ACT before the state_v wait: f2x's Exps otherwise
                    # queue behind proj-V steps and the final units stall on
                    # the Exp chain; f3x spreads between the first units so
                    # the po-psum recycle stays paced
                    fill(2, 0); yield
                    fill(2, 1); yield
                    while ci not in state_v:
                        yield
                    v_sb = state_v.pop(ci)
                    vm = memp["vm"]
                    state_v_lo.pop(ci, None)
                    unit(0); yield
                    fill(3, 0); yield
                    unit(1); out_dma(0); yield
                    fill(3, 1); yield
                    unit(2); yield
                    unit(3); out_dma(1); yield
                else:
                    while ci not in state_v:
                        yield
                    v_sb = state_v.pop(ci)
                    vm = memp["vm"]
                    state_v_lo.pop(ci, None)
                    unit(0); yield
                    unit(1); out_dma(0); yield
                    fill(2, 0); yield
                    fill(2, 1); yield
                    unit(2); yield
                    unit(3); out_dma(1); yield
                    fill(3, 0); yield
                    fill(3, 1); yield
                unit(4); yield
                unit(5); out_dma(2); yield
                # last pair: defer unit 6's normalize past unit 7's matmuls
                # and reciprocal so only recip7+mult7 trail the PE stream
                ps6 = unit_mm(6)
                rec6 = unit_recip(6, ps6); yield
                ps7 = unit_mm(7)
                unit_mult(6, ps6, rec6)
                rec7 = unit_recip(7, ps7)
                unit_mult(7, ps7, rec7)
                out_dma(3)

            def drain(*gens):
                gens = [g for g in gens if g is not None]
                while gens:
                    nxt = []
                    for g in gens:
                        try:
                            next(g)
                            nxt.append(g)
                        except StopIteration:
                            pass
                    gens = nxt

            # 3-way weave: chunk ci's attention runs with chunk ci+1's
            # projections, and attn(ci+1) joins early (it self-waits on
            # its state) so the attention tail always has matmul filler
            attns_g = [attn_steps(ci) for ci in range(CPC)]
            p0 = proj_steps(0)
            gens0 = [p0, attns_g[0]]
            must0 = {id(p0)}
            while must0:
                for g in list(gens0):
                    try:
                        next(g)
                    except StopIteration:
                        gens0.remove(g)
                        must0.discard(id(g))
            for ci in range(CPC):
                gens = [attns_g[ci]]
                must = {id(attns_g[ci])}
                if ci + 1 < CPC:
                    pj = proj_steps(ci + 1)
                    # proj(ci+1) and attn(ci+1) step twice per round so the
                    # next chunk's projections and fills/Exps run far enough
                    # ahead that the closing PV units never wait on an Exp
                    # (matters most for the last chunk, which runs with no
                    # projection filler)
                    gens = [pj, attns_g[ci],
                            attns_g[ci + 1], attns_g[ci + 1]]
                    must.add(id(pj))
                while must:
                    for g in list(gens):
                        try:
                            next(g)
                        except StopIteration:
                            if g in gens:
                                gens.remove(g)
                            must.discard(id(g))

    nc.compile()
    return nc


_NC_CACHE = None


def kernel(hidden_states, attention_mask, self_memory, Wq, bq, Wk, bk, Wv, bv):
    global _NC_CACHE, LAST_RESULTS
    hidden_states = np.asarray(np.asarray(hidden_states), np.float32)
    attention_mask = np.asarray(np.asarray(attention_mask), np.float32)
    self_memory = np.asarray(np.asarray(self_memory), np.float32)
    wqT = np.ascontiguousarray(
        (np.asarray(Wq, np.float32).T * 0.125).astype(np.float16)
    )
    wkT = np.ascontiguousarray(np.asarray(Wk, np.float32).T.astype(np.float16))
    wvT = np.ascontiguousarray(np.asarray(Wv, np.float32).T.astype(np.float16))
    bqv = np.ascontiguousarray(
        np.asarray(bq, np.float32).reshape(NJT, 128).T * 0.125
    )
    bkv = np.ascontiguousarray(np.asarray(bk, np.float32).reshape(NJT, 128).T)
    bvbr = np.ascontiguousarray(np.asarray(bv, np.float32).reshape(1, D))
    bvbr16 = bvbr.astype(np.float16)

    # additive mask along the key axis, per (b, c): [yt0 | yt1 | memory].
    # Clamped to -11: softmax is shift-invariant, so for a fully-masked
    # chunk exp(s - 11) still normalizes to softmax(s) (matching the
    # reference) instead of underflowing fp16 to 0/0; for partial masks
    # the e^-11 leakage per masked key is ~2e-5 of a valid key.
    am = np.maximum(attention_mask.reshape(B, C, W), -11.0)
    chunk_has_valid = (attention_mask.reshape(B, C, W) == 0.0).sum(axis=2) > 0
    # the reference broadcasts mem_mask[:, None, None, :] over the
    # memory-TOKEN axis (M == C): mem token j is masked for every chunk
    # of batch b iff chunk j of batch b is fully masked. As a per-em-
    # partition bias vector: token j sits at partitions {32*rb + j}.
    mem_mask = np.where(chunk_has_valid, 0.0, -11.0).astype(np.float32)  # [B, C=M]
    memv = np.zeros((B, 128), np.float32)
    for rb in range(4):
        memv[:, 32 * rb:32 * rb + M] = mem_mask

    if _NC_CACHE is None:
        _NC_CACHE = _build_kernel()
    nc = _NC_CACHE

    x16 = hidden_states.astype(np.float16)
    mem16 = self_memory.astype(np.float16)

    in_maps = []
    for core in range(N_CORES):
        b = core // (N_CORES // B)
        c0 = (core % (N_CORES // B)) * CPC
        mvT = np.zeros((128, CPC * 3), np.float32)
        for ci in range(CPC):
            mvT[:, ci * 3 + 0] = am[b, c0 + ci, 0:128]
            mvT[:, ci * 3 + 1] = am[b, c0 + ci, 128:256]
            mvT[:, ci * 3 + 2] = memv[b]
        # feature-major pre-transposed x: [128, NJT, TPC]
        xT = np.ascontiguousarray(
            x16[b, c0 * W:(c0 + CPC) * W, :]
            .T.reshape(NJT, 128, TPC).transpose(1, 0, 2)
        ).reshape(128, NJT * TPC)
        memT = np.ascontiguousarray(
            mem16[b].T.reshape(NJT, 128, M).transpose(1, 0, 2)
        ).reshape(128, NJT * M)
        in_maps.append(
            {
                "xT": xT,
                "memT": memT,
                "wqT": wqT,
                "wkT": wkT,
                "wvT": wvT,
                "bqv": bqv,
                "bkv": bkv,
                "bvbr": bvbr,
                "bvbr16": bvbr16,
                "maskvT": mvT,
            }
        )

    res = run_bass_kernel_spmd(nc, in_maps, list(range(N_CORES)), trace=TRACE)
    LAST_RESULTS = res

    out = np.empty((B, S, D), np.float32)
    for core in range(N_CORES):
        b = core // (N_CORES // B)
        c0 = (core % (N_CORES // B)) * CPC
        out[b, c0 * W:(c0 + CPC) * W, :] = res.results[core]["out"].astype(
            np.float32
        )
    return out



# revision 67
# speedup vs baseline: 1.0071x; 1.0039x over previous
"""Bass/Trainium2 kernel for chunked local attention with memory tokens
(BertSelfAttention variant). Self-contained: hardcodes all shapes.

Sharding: 8 cores, each handles 4 of the 32 (batch, chunk) pairs.
  core i -> b = i // 4, chunks 4*(i % 4) .. 4*(i % 4) + 3
No collectives; weights replicated per core; host scatters/gathers.

Per-core device computation (PE operands fp16, accumulation fp32):
  - xT [128, 8, 1024] arrives pre-transposed from host (feature-major),
    sliced per chunk; no device transposes
  - qT[jt] = x@(Wq/8) + bq/8 (feature-major)   kT[jt] = x@Wk + bk
  - v[t, yt, h, 65] token-major with a ones column at 64 that makes the
    PV matmul emit softmax denominators
  - memory-token K is materialized block-diagonally (kTm3 [128, jt, 64]:
    rows 0:64 x cols 0:16 = even head, rows 64:128 x cols 32:48 = odd) so
    one 128-contraction matmul scores 2 heads' mem keys at 32-aligned
    psum partitions; all 16 heads' mem scores share one 2-bank psum and
    a single Exp
  - local scoresT per (head-quad, yt) fill a 2-bank [128, 1024] psum
    (both 64-row ab halves, quadrant-packed) -> one Exp per fill
  - probs = Exp(scoresT + mask[y]) in fp16; additive key mask rides the
    ACT bias operand (per-partition scalar)
  - out_unnorm[x, 4*(hd|den)] per head-pair in one psum bank; strided
    reciprocal + one broadcast multiply normalize and emit fp16 out
Output fp16 on device, upcast to fp32 on host.

Scheduling: generator-based software pipelining weaves chunk ci's
attention with chunk ci+1's projections (attn(ci+1) steps twice per
round so its fills/Exps run far ahead) so the PE streams matmuls while
ACT runs the Exps; tiny warm matmuls keep the PE p-state ramp clock
alive across startup DMA waits.

DMA orchestration (the cost model charges ~625ns serialized HWDGE issue
+ serial transfers on one DMA-engine set + 900ns sem per DMA):
  - Q/K weights stream on the sync queue as 256KB granules in the exact
    order chunk-0's psum fills consume them, with x(0) halves
    interleaved at first use -> first projection matmul at ~4.4us
    (DMA-pipe floor) instead of ~7.8us
  - small constants (biases, mask, mem tokens) go through the Pool
    engine's SWDGE path, off the HWDGE critical chain; the V bias
    broadcast [128, D] is a DMA (no PE/DVE work)
  - mem-V vm: bias folded into the psum as a rank-1 (bvrow^T @ ones)
    matmul, ACT copies psum->fp16 (Pool cannot touch PSUM), and the
    head-major relayout + 4 partition-base replicas come straight off
    the transposed buffer as DMAs (no DVE hop)
  - the last chunk defers unit 6's normalize past unit 7's matmuls so
    only recip7+mult7+one 256KB DMA trail the final PE instruction
"""

import sys

sys.path.insert(0, "/opt/trn_rl_repo")

import numpy as np

import concourse.tile as tile
from concourse import bacc, mybir
from concourse.bass_utils import run_bass_kernel_spmd

F32 = mybir.dt.float32
F16 = mybir.dt.float16

B, S, D = 2, 4096, 1024
H, HD = 16, 64
W = 256            # attention window (chunk length)
C = S // W         # 16 chunks
M = 16             # memory tokens
N_CORES = 8
CPC = C * B // N_CORES  # 4 chunks per core
TPC = CPC * W           # 1024 chunk tokens per core
NJT = D // 128          # 8 feature tiles
VW = HD + 1             # v width (64 hd + ones col)

TRACE = False
LAST_RESULTS = None


def _mem_slot(h):
    """(partition base, free block) of head h's mem scores / probs."""
    jt, u = h // 2, h % 2
    return 64 * (jt % 2) + 32 * u, jt // 2


def _build_kernel():
    nc = bacc.Bacc(None, target_bir_lowering=False)

    xT_d = nc.declare_dram_parameter("xT", [128, NJT * TPC], F16, isOutput=False)
    memT_d = nc.declare_dram_parameter("memT", [128, NJT * M], F16, isOutput=False)
    wqT_d = nc.declare_dram_parameter("wqT", [D, D], F16, isOutput=False)
    wkT_d = nc.declare_dram_parameter("wkT", [D, D], F16, isOutput=False)
    wvT_d = nc.declare_dram_parameter("wvT", [D, D], F16, isOutput=False)
    bq_d = nc.declare_dram_parameter("bqv", [128, NJT], F32, isOutput=False)
    bk_d = nc.declare_dram_parameter("bkv", [128, NJT], F32, isOutput=False)
    bvr_d = nc.declare_dram_parameter("bvbr", [1, D], F32, isOutput=False)
    bvr16_d = nc.declare_dram_parameter("bvbr16", [1, D], F16, isOutput=False)
    msk_d = nc.declare_dram_parameter("maskvT", [128, CPC * 3], F32, isOutput=False)
    out_d = nc.declare_dram_parameter("out", [TPC, D], F16, isOutput=True)

    with tile.TileContext(nc) as tc:
        with (
            tc.tile_pool(name="const", bufs=1) as cpool,
            tc.tile_pool(name="wpool", bufs=1) as wpool,
            tc.tile_pool(name="xtpool", bufs=4) as xtpool,
            tc.tile_pool(name="qkpool", bufs=16) as qkpool,
            tc.tile_pool(name="vpool", bufs=4) as vpool,
            tc.tile_pool(name="epool", bufs=16) as epool,
            tc.tile_pool(name="empool", bufs=3) as empool,
            tc.tile_pool(name="opool", bufs=4) as opool,
            tc.tile_pool(name="rpool", bufs=4) as rpool,
            tc.tile_pool(name="pp", bufs=2, space="PSUM") as pp_pool,
            tc.tile_pool(name="ps", bufs=2, space="PSUM") as ps_pool,
            tc.tile_pool(name="po", bufs=2, space="PSUM") as po_pool,
        ):
            x_tiles = {}

            def load_x(ci, eng, split=False):
                x_t = xtpool.tile([128, NJT, W], F16, tag="xT", name="xT")
                xs = xT_d.rearrange("p (o t) -> p o t", t=TPC)[
                    :, :, ci * W:(ci + 1) * W
                ]
                if split:
                    eng.dma_start(x_t[:, 0:4, :], xs[:, 0:4, :])
                    eng.dma_start(x_t[:, 4:8, :], xs[:, 4:8, :])
                else:
                    eng.dma_start(x_t[:], xs)
                x_tiles[ci] = [x_t[:, d, :] for d in range(NJT)]

            w_all = wpool.tile([128, 3 * NJT, D], F16, tag="w_all")

            def load_w_piece(wi, wd, o0, og, c0, cw, eng):
                eng.dma_start(
                    w_all[:, wi * NJT + o0: wi * NJT + o0 + og, c0:c0 + cw],
                    wd.rearrange("(o p) c -> p o c", p=128)[
                        :, o0:o0 + og, c0:c0 + cw
                    ],
                )

            def wq(d):
                return w_all[:, d, :]

            def wk(d):
                return w_all[:, NJT + d, :]

            def wv(d):
                return w_all[:, 2 * NJT + d, :]

            # ---- pool-queue constants (SWDGE path, bypasses the HWDGE
            # serialization on the sync queue): ones first so the PE warm
            # matmul fires ~250ns in, then small constants in consumption
            # order, then the V weights (needed only mid-chunk-0).
            ones1 = cpool.tile([1, 128], F16, tag="ones1")
            nc.gpsimd.memset(ones1[:], 1.0)
            bqv = cpool.tile([128, NJT], F32, tag="bqv")
            nc.gpsimd.dma_start(bqv[:], bq_d[:])
            bkv = cpool.tile([128, NJT], F32, tag="bkv")
            nc.gpsimd.dma_start(bkv[:], bk_d[:])
            mskv = cpool.tile([128, CPC * 3], F32, tag="mskv")
            nc.gpsimd.dma_start(mskv[:], msk_d[:])
            xTm = cpool.tile([128, NJT, M], F16, tag="xTm")
            nc.gpsimd.dma_start(xTm[:], memT_d.rearrange("p (o m) -> p o m", m=M))
            bvrow = cpool.tile([1, D], F16, tag="bvrow")
            nc.gpsimd.dma_start(bvrow[:], bvr16_d[:])
            bvb = cpool.tile([128, D], F32, tag="bvb")

            # ---- sync-queue streams: 128KB weight granules in the exact
            # order chunk-0's psum fills consume them (jp-pair cols x d-half
            # rows), with x(0) halves interleaved at their first use.
            x_t0 = xtpool.tile([128, NJT, W], F16, tag="xT", name="xT")
            xs0 = xT_d.rearrange("p (o t) -> p o t", t=TPC)[:, :, 0:W]
            nc.sync.dma_start(x_t0[:, 0:4, :], xs0[:, 0:4, :])
            x_tiles[0] = [x_t0[:, d, :] for d in range(NJT)]
            # 256KB granules in chunk-0 consumption order (jp-pair cols x
            # d-half rows)
            GRAN = ((0, 0), (0, 256), (4, 0), (4, 256),
                    (0, 512), (0, 768), (4, 512), (4, 768))
            for o0, c0 in GRAN[:2]:
                load_w_piece(0, wqT_d, o0, 4, c0, 256, nc.sync)
            nc.sync.dma_start(x_t0[:, 4:8, :], xs0[:, 4:8, :])
            for o0, c0 in GRAN[2:]:
                load_w_piece(0, wqT_d, o0, 4, c0, 256, nc.sync)
            for o0, c0 in GRAN:
                load_w_piece(1, wkT_d, o0, 4, c0, 256, nc.sync)
            load_w_piece(2, wvT_d, 0, 4, 0, D, nc.sync)
            load_w_piece(2, wvT_d, 4, 4, 0, D, nc.sync)
            nc.sync.dma_start(bvb[:], bvr_d[:].to_broadcast((128, D)))
            load_x(1, nc.sync)
            load_x(2, nc.sync)
            load_x(3, nc.sync)

            # tiny dep-free matmuls: start the PE p-state ramp clock early
            # (ones lands ~250ns via the pool memset) and refresh it off the
            # small pool DMAs so the idle gaps before the first projection
            # fills stay under the ~3us p-state reset threshold
            ps_warm = pp_pool.tile([128, 512], F32, tag="pp")
            nc.tensor.matmul(
                ps_warm[:16, :16], ones1[:, :16], ones1[:, :16],
                start=True, stop=True,
            )
            nc.tensor.matmul(
                ps_warm[:4, 16:20], bqv[:16, :4], bqv[:16, :4],
                start=True, stop=True,
            )
            x0d0 = x_tiles[0][0]
            nc.tensor.matmul(
                ps_warm[:16, 32:48], x0d0[:16, :16], x0d0[:16, :16],
                start=True, stop=True,
            )

            # ---- memory tokens (emitted inside chunk 0's flow) ----
            memp = {}

            def emit_ktm():
                # block-diagonal mem-K: [128, jt, 64]; even head rows 0:64 ->
                # cols 0:16, odd head rows 64:128 -> cols 32:48, rest zero
                kTm3 = cpool.tile([128, NJT, 64], F16, tag="kTm3", name="kTm3")
                nc.gpsimd.memset(kTm3[:], 0.0)
                ps_k = pp_pool.tile([128, 512], F32, tag="pp", name="ps_ktm")
                for jt in range(NJT):
                    for d in range(NJT):
                        nc.tensor.matmul(
                            ps_k[:, jt * M:(jt + 1) * M],
                            wk(d)[:, jt * 128:(jt + 1) * 128],
                            xTm[:, d, :],
                            start=(d == 0), stop=(d == NJT - 1),
                            skip_group_check=True,
                        )
                for u in range(2):
                    nc.vector.tensor_tensor(
                        kTm3[64 * u:64 * (u + 1), :, 32 * u:32 * u + 16],
                        ps_k[64 * u:64 * (u + 1), :NJT * M].rearrange(
                            "p (j m) -> p j m", m=M
                        ),
                        bkv[64 * u:64 * (u + 1), :][:, :, None].to_broadcast(
                            (64, NJT, M)
                        ),
                        mybir.AluOpType.add,
                    )
                memp["kTm3"] = kTm3

            def emit_vm():
                # mem-V computed feature-major in one cheap psum fill, then
                # token-major via an xbar DMA-transpose of the m-padded
                # [128, jt, 128] layout (dst[p, jt, f] = src[f, jt*128+p]);
                # replicated at partition bases 0/32/64/96 for the
                # 32-aligned mem-PV stationaries
                vm = cpool.tile([128, M, VW], F16, tag="vm", name="vm")
                for rb in range(4):
                    nc.gpsimd.memset(vm[32 * rb:32 * rb + M, :, HD:HD + 1], 1.0)
                vmT = cpool.tile([128, NJT, 128], F16, tag="vmT", name="vmT")
                nc.gpsimd.memset(vmT[:], 0.0)
                ps_t = pp_pool.tile([128, 512], F32, tag="pp", name="ps_vm")
                for jt in range(NJT):
                    for d in range(NJT):
                        nc.tensor.matmul(
                            ps_t[:, jt * M:(jt + 1) * M],
                            wv(d)[:, jt * 128:(jt + 1) * 128],
                            xTm[:, d, :],
                            start=(d == 0), stop=False,
                            skip_group_check=True,
                        )
                    # fold the V bias in as a rank-1 update (bvrow^T @ ones)
                    # so no DVE/ACT bias-add is needed before the transpose
                    nc.tensor.matmul(
                        ps_t[:, jt * M:(jt + 1) * M],
                        bvrow[:, jt * 128:(jt + 1) * 128],
                        ones1[:, :M],
                        start=False, stop=True,
                        skip_group_check=True,
                    )
                # psum -> fp16 on ACT (pool can't touch PSUM; DVE is busy
                # with projection bias-adds around this point)
                nc.scalar.activation(
                    vmT[:, :, :M],
                    ps_t[:, :NJT * M].rearrange("p (j m) -> p j m", m=M),
                    mybir.ActivationFunctionType.Copy,
                )
                vmB = cpool.tile([128, NJT, 128], F16, tag="vmB", name="vmB")
                nc.sync.dma_start_transpose(
                    vmB[:], vmT[:].rearrange("p j m -> p (j m)")
                )
                # head-major relayout + partition-base replication straight
                # from the transposed buffer (no DVE hop)
                for rb in range(4):
                    nc.sync.dma_start(
                        vm[32 * rb:32 * rb + M, :, :HD].rearrange(
                            "m (j u) f -> m j u f", u=2
                        ),
                        vmB[:M, :, :].rearrange("m j (u f) -> m j u f", u=2),
                    )
                memp["vm"] = vm

            # ---- per-chunk phases as generators; the main loop weaves
            # chunk ci's attention with chunk ci+1's projections so the PE
            # always has projection matmuls to stream while Act runs Exps.
            # The handoff is split Q/K vs V: score fills only need Q/K, so
            # they start a V-phase early and V matmuls fill attention tails.
            state_qk = {}
            state_v = {}
            state_v_lo = {}  # heads 0:8 complete (units 0-3 can start)
            qk_pairs = {}   # (ci, jp) -> (q pair tile, k pair tile)

            def proj_steps(ci):
                xT = x_tiles.pop(ci)
                qT, kT = [], []
                for which, wfn, bias, lst, tg in (
                    (0, wq, bqv, qT, "qT"),
                    (1, wk, bkv, kT, "kT"),
                ):
                    # pp-buf pairs of jt-pair fills, split at the 1MB
                    # weight-DMA boundary (d 0-3 | 4-7) so chunk 0's
                    # matmuls track weight arrival
                    for hw in range(2):
                        pss = []
                        for j2 in range(2):
                            jp = 2 * hw + j2
                            ps_q = pp_pool.tile(
                                [128, 512], F32, tag="pp", name="ps_q"
                            )
                            for u in range(2):
                                jt = 2 * jp + u
                                for d in range(NJT // 2):
                                    nc.tensor.matmul(
                                        ps_q[:, u * 256:(u + 1) * 256],
                                        wfn(d)[:, jt * 128:(jt + 1) * 128],
                                        xT[d][:],
                                        start=(u == 0 and d == 0),
                                        stop=False,
                                    )
                            pss.append(ps_q)
                        for j2 in range(2):
                            jp = 2 * hw + j2
                            ps_q = pss[j2]
                            for u in range(2):
                                jt = 2 * jp + u
                                for d in range(NJT // 2, NJT):
                                    nc.tensor.matmul(
                                        ps_q[:, u * 256:(u + 1) * 256],
                                        wfn(d)[:, jt * 128:(jt + 1) * 128],
                                        xT[d][:],
                                        start=False,
                                        stop=(u == 1 and d == NJT - 1),
                                    )
                            pair_t = qkpool.tile(
                                [128, 2, W], F16, tag=tg, name=f"pair_{tg}"
                            )
                            nc.vector.tensor_tensor(
                                pair_t[:],
                                ps_q[:].rearrange("p (u t) -> p u t", u=2),
                                bias[:, 2 * jp:2 * jp + 2][:, :, None]
                                .to_broadcast((128, 2, W)),
                                mybir.AluOpType.add,
                            )
                            lst.append(pair_t)
                            if which == 1:
                                # publish the (q, k) head-quad as soon as its
                                # K pair lands so the next chunk's fills (and
                                # their Exps) start mid-projection instead of
                                # after the whole K section
                                qk_pairs[(ci, jp)] = (qT[jp], pair_t)
                            yield
                    if ci == 0 and which == 1:
                        emit_ktm()
                        yield
                state_qk[ci] = (
                    [qT[jt // 2][:, jt % 2, :] for jt in range(NJT)],
                    [kT[jt // 2][:, jt % 2, :] for jt in range(NJT)],
                )

                # V projection (token-major fp16, heads + ones col)
                v_sb = vpool.tile([128, 2, H, VW], F16, tag="v_sb")
                nc.gpsimd.memset(v_sb[:, :, :, HD:HD + 1], 1.0)
                for tt in range(2):
                    pss = []
                    for half in range(2):
                        ps_v = pp_pool.tile([128, 512], F32, tag="pp")
                        for d in range(NJT // 2):
                            nc.tensor.matmul(
                                ps_v[:], xT[d][:, tt * 128:(tt + 1) * 128],
                                wv(d)[:, half * 512:(half + 1) * 512],
                                start=(d == 0), stop=False,
                            )
                        pss.append(ps_v)
                        if ci == CPC - 1:
                            # finer V yields for the last projection so the
                            # last chunk's fills slot in at ~850ns cadence
                            # and its Exp chain never starves
                            yield
                    for half in range(2):
                        ps_v = pss[half]
                        for d in range(NJT // 2, NJT):
                            nc.tensor.matmul(
                                ps_v[:], xT[d][:, tt * 128:(tt + 1) * 128],
                                wv(d)[:, half * 512:(half + 1) * 512],
                                start=False, stop=(d == NJT - 1),
                            )
                        nc.vector.tensor_tensor(
                            v_sb[:, tt, half * 8:(half + 1) * 8, :HD],
                            ps_v[:].rearrange("p (h f) -> p h f", h=8),
                            bvb[:, half * 512:(half + 1) * 512].rearrange(
                                "p (h f) -> p h f", h=8
                            ),
                            mybir.AluOpType.add,
                        )
                        if tt == 1 and half == 0:
                            # heads 0:8 complete for both token halves:
                            # units 0-3 can start under the V tail
                            state_v_lo[ci] = v_sb
                        yield
                    if ci == 0 and tt == 0:
                        emit_vm()
                state_v[ci] = v_sb

            def attn_steps(ci):
                last = ci == CPC - 1

                # mem scores: all 16 heads in one 2-bank psum, one Exp.
                # Head pair jt lands at 32-aligned partition bases via the
                # block-diagonal stationary and tile_position cols.
                def emit_ms():
                    qT, kT = state_qk.pop(ci)
                    kTm3 = memp["kTm3"]
                    ps_m = ps_pool.tile([128, 1024], F32, tag="ps", name="ps_ms")
                    for jt in range(NJT):
                        c0 = 64 * (jt % 2)
                        g = jt // 2
                        nc.tensor.matmul(
                            ps_m[c0:c0 + 64, g * 256:(g + 1) * 256],
                            kTm3[:, jt, :],
                            qT[jt][:],
                            start=True, stop=True,
                            tile_position=(0, c0),
                            skip_group_check=True,
                        )
                    em = empool.tile([128, 4, 256], F16, tag="em", name="em")
                    nc.scalar.activation(
                        em[:], ps_m[:].rearrange("p (g t) -> p g t", g=4),
                        mybir.ActivationFunctionType.Exp,
                        bias=mskv[:, ci * 3 + 2: ci * 3 + 3],
                    )
                    return em

                out_sb = opool.tile([128, 2, D], F16, tag="out_sb", name="out_sb")
                eloc = {}   # (hpq, yt) -> [128, 1024] fp16

                def fill(hpq, yt):
                    # local scoresT for 4 heads (one quad, one key half);
                    # both 64-row ab halves quadrant-packed; one Exp
                    qp, kp = qk_pairs[(ci, hpq)]
                    ps_s = ps_pool.tile([128, 1024], F32, tag="ps", name="ps_s")
                    for ab in range(2):
                        p0 = 64 * ab
                        for u in range(2):
                            nc.tensor.matmul(
                                ps_s[:, ab * 512 + u * 256:
                                     ab * 512 + (u + 1) * 256],
                                kp[p0:p0 + 64, u, yt * 128:(yt + 1) * 128],
                                qp[p0:p0 + 64, u, :],
                                start=(u == 0), stop=(u == 1),
                                tile_position=(p0, 0),
                                skip_group_check=True,
                            )
                    e_t = epool.tile([128, 1024], F16, tag="exps")
                    nc.scalar.activation(
                        e_t[:], ps_s[:],
                        mybir.ActivationFunctionType.Exp,
                        bias=mskv[:, ci * 3 + yt: ci * 3 + yt + 1],
                    )
                    eloc[(hpq, yt)] = e_t

                def unit_mm(hp, wide=False):
                    # PV for head pair hp: one psum bank, 4 blocks (h, xb)
                    # of 65 (64 hd + denom). wide=True borrows a (free)
                    # fill-psum tile so the last chunk's closing unit train
                    # has 4 psum buffers and never stalls on normalize
                    # recycling.
                    if wide:
                        ps_w = ps_pool.tile([128, 1024], F32, tag="ps",
                                            name="ps_w")
                        ps_o = ps_w[:, :4 * VW]
                    else:
                        ps_o = po_pool.tile([128, 4 * VW], F32, tag="po")
                    for ab in range(2):
                        h = 2 * hp + ab
                        e0 = eloc[(hp // 2, 0)]
                        e1 = eloc[(hp // 2, 1)]
                        base, g = _mem_slot(h)
                        for xb in range(2):
                            o = (ab * 2 + xb) * VW
                            xs = (h % 2) * 512 + (hp % 2) * 256 + xb * 128
                            nc.tensor.matmul(
                                ps_o[:, o:o + VW],
                                e0[:, xs:xs + 128], v_sb[:, 0, h, :],
                                start=True, stop=False,
                            )
                            nc.tensor.matmul(
                                ps_o[:, o:o + VW],
                                e1[:, xs:xs + 128], v_sb[:, 1, h, :],
                                start=False, stop=False,
                            )
                            nc.tensor.matmul(
                                ps_o[:, o:o + VW],
                                em[base:base + M, g, xb * 128:(xb + 1) * 128],
                                vm[base:base + M, h, :],
                                start=False, stop=True,
                                tile_position=(base, 0),
                            )
                    return ps_o

                def unit_recip(hp, ps_o):
                    rec = rpool.tile([128, 4], F32, tag="rec", name="rec")
                    nc.vector.reciprocal(
                        rec[:].rearrange("p (k o) -> p k o", o=1),
                        ps_o[:].rearrange("p (k w) -> p k w", w=VW)[
                            :, :, HD:HD + 1
                        ],
                    )
                    return rec

                def unit_mult(hp, ps_o, rec):
                    nc.vector.tensor_tensor(
                        out_sb[:, :, 2 * hp * HD:(2 * hp + 2) * HD].rearrange(
                            "p x (a f) -> p a x f", a=2
                        ),
                        ps_o[:].rearrange("p (a x w) -> p a x w", a=2, x=2)[
                            :, :, :, :HD
                        ],
                        rec[:].rearrange("p (a x) -> p a x", a=2)[
                            :, :, :, None
                        ].to_broadcast((128, 2, 2, HD)),
                        mybir.AluOpType.mult,
                    )

                def unit(hp):
                    ps_o = unit_mm(hp)
                    unit_mult(hp, ps_o, unit_recip(hp, ps_o))

                def out_dma(qtr):
                    nc.sync.dma_start(
                        out_d.rearrange("(x p) c -> p x c", p=128)[
                            :, 2 * ci:2 * ci + 2, qtr * 256:(qtr + 1) * 256
                        ],
                        out_sb[:, :, qtr * 256:(qtr + 1) * 256],
                    )

                # fills gate on their own (q, k) head-pair so they start
                # mid-K-projection; Exps spread forward on ACT. The last
                # chunk front-loads ALL fills (spaced a round apart for psum
                # recycling) so its Exps finish during proj(last)'s V section
                # and the closing PV units never wait on ACT.
                while ci not in state_qk:
                    yield
                fill(0, 0); yield
                fill(0, 1); yield
                em = emit_ms(); yield
                fill(1, 0); yield
                fill(1, 1); yield
                if last:
                    # feed ACT before the state_v wait: f2x's Exps otherwise
                    # queue behind proj-V steps and the final units stall on
                    # the Exp chain; f3x spreads between the first units so
                    # the po-psum recycle stays paced
                    fill(2, 0); yield
                    fill(2, 1); yield
                    while ci not in state_v:
                        yield
                    v_sb = state_v.pop(ci)
                    vm = memp["vm"]
                    state_v_lo.pop(ci, None)
                    unit(0); yield
                    fill(3, 0); yield
                    unit(1); out_dma(0); yield
                    fill(3, 1); yield
                    unit(2); yield
                    unit(3); out_dma(1); yield
                else:
                    while ci not in state_v:
                        yield
                    v_sb = state_v.pop(ci)
                    vm = memp["vm"]
                    state_v_lo.pop(ci, None)
                    unit(0); yield
                    unit(1); out_dma(0); yield
                    fill(2, 0); yield
                    fill(2, 1); yield
                    unit(2); yield
                    unit(3); out_dma(1); yield
                    fill(3, 0); yield
                    fill(3, 1); yield
                if last:
                    # units 4/5 borrow the (now free) fill-psum tiles so the
                    # closing train runs at PE rate with DVE trailing
                    ps4 = unit_mm(4, wide=True)
                    unit_mult(4, ps4, unit_recip(4, ps4)); yield
                    ps5 = unit_mm(5, wide=True)
                    unit_mult(5, ps5, unit_recip(5, ps5))
                    out_dma(2); yield
                else:
                    unit(4); yield
                    unit(5); out_dma(2); yield
                # last pair: defer unit 6's normalize past unit 7's matmuls
                # and reciprocal so only recip7+mult7 trail the PE stream
                ps6 = unit_mm(6)
                rec6 = unit_recip(6, ps6); yield
                ps7 = unit_mm(7)
                unit_mult(6, ps6, rec6)
                rec7 = unit_recip(7, ps7)
                unit_mult(7, ps7, rec7)
                out_dma(3)

            def drain(*gens):
                gens = [g for g in gens if g is not None]
                while gens:
                    nxt = []
                    for g in gens:
                        try:
                            next(g)
                            nxt.append(g)
                        except StopIteration:
                            pass
                    gens = nxt

            # 3-way weave: chunk ci's attention runs with chunk ci+1's
            # projections, and attn(ci+1) joins early (it self-waits on
            # its state) so the attention tail always has matmul filler
            attns_g = [attn_steps(ci) for ci in range(CPC)]
            p0 = proj_steps(0)
            gens0 = [p0, attns_g[0]]
            must0 = {id(p0)}
            while must0:
                for g in list(gens0):
                    try:
                        next(g)
                    except StopIteration:
                        gens0.remove(g)
                        must0.discard(id(g))
            for ci in range(CPC):
                gens = [attns_g[ci]]
                must = {id(attns_g[ci])}
                if ci + 1 < CPC:
                    pj = proj_steps(ci + 1)
                    # proj(ci+1) and attn(ci+1) step twice per round so the
                    # next chunk's projections and fills/Exps run far enough
                    # ahead that the closing PV units never wait on an Exp
                    # (matters most for the last chunk, which runs with no
                    # projection filler)
                    gens = [pj, attns_g[ci],
                            attns_g[ci + 1], attns_g[ci + 1]]
                    must.add(id(pj))
                while must:
                    for g in list(gens):
                        try:
                            next(g)
                        except StopIteration:
                            if g in gens:
                                gens.remove(g)
                            must.discard(id(g))

    nc.compile()
    return nc


_NC_CACHE = None


def kernel(hidden_states, attention_mask, self_memory, Wq, bq, Wk, bk, Wv, bv):
    global _NC_CACHE, LAST_RESULTS
    hidden_states = np.asarray(np.asarray(hidden_states), np.float32)
    attention_mask = np.asarray(np.asarray(attention_mask), np.float32)
    self_memory = np.asarray(np.asarray(self_memory), np.float32)
    wqT = np.ascontiguousarray(
        (np.asarray(Wq, np.float32).T * 0.125).astype(np.float16)
    )
    wkT = np.ascontiguousarray(np.asarray(Wk, np.float32).T.astype(np.float16))
    wvT = np.ascontiguousarray(np.asarray(Wv, np.float32).T.astype(np.float16))
    bqv = np.ascontiguousarray(
        np.asarray(bq, np.float32).reshape(NJT, 128).T * 0.125
    )
    bkv = np.ascontiguousarray(np.asarray(bk, np.float32).reshape(NJT, 128).T)
    bvbr = np.ascontiguousarray(np.asarray(bv, np.float32).reshape(1, D))
    bvbr16 = bvbr.astype(np.float16)

    # additive mask along the key axis, per (b, c): [yt0 | yt1 | memory].
    # Clamped to -11: softmax is shift-invariant, so for a fully-masked
    # chunk exp(s - 11) still normalizes to softmax(s) (matching the
    # reference) instead of underflowing fp16 to 0/0; for partial masks
    # the e^-11 leakage per masked key is ~2e-5 of a valid key.
    am = np.maximum(attention_mask.reshape(B, C, W), -11.0)
    chunk_has_valid = (attention_mask.reshape(B, C, W) == 0.0).sum(axis=2) > 0
    # the reference broadcasts mem_mask[:, None, None, :] over the
    # memory-TOKEN axis (M == C): mem token j is masked for every chunk
    # of batch b iff chunk j of batch b is fully masked. As a per-em-
    # partition bias vector: token j sits at partitions {32*rb + j}.
    mem_mask = np.where(chunk_has_valid, 0.0, -11.0).astype(np.float32)  # [B, C=M]
    memv = np.zeros((B, 128), np.float32)
    for rb in range(4):
        memv[:, 32 * rb:32 * rb + M] = mem_mask

    if _NC_CACHE is None:
        _NC_CACHE = _build_kernel()
    nc = _NC_CACHE

    x16 = hidden_states.astype(np.float16)
    mem16 = self_memory.astype(np.float16)

    in_maps = []
    for core in range(N_CORES):
        b = core // (N_CORES // B)
        c0 = (core % (N_CORES // B)) * CPC
        mvT = np.zeros((128, CPC * 3), np.float32)
        for ci in range(CPC):
            mvT[:, ci * 3 + 0] = am[b, c0 + ci, 0:128]
            mvT[:, ci * 3 + 1] = am[b, c0 + ci, 128:256]
            mvT[:, ci * 3 + 2] = memv[b]
        # feature-major pre-transposed x: [128, NJT, TPC]
        xT = np.ascontiguousarray(
            x16[b, c0 * W:(c0 + CPC) * W, :]
            .T.reshape(NJT, 128, TPC).transpose(1, 0, 2)
        ).reshape(128, NJT * TPC)
        memT = np.ascontiguousarray(
            mem16[b].T.reshape(NJT, 128, M).transpose(1, 0, 2)
        ).reshape(128, NJT * M)
        in_maps.append(
            {
                "xT": xT,
                "memT": memT,
                "wqT": wqT,
                "wkT": wkT,
                "wvT": wvT,
                "bqv": bqv,
                "bkv": bkv,
                "bvbr": bvbr,
                "bvbr16": bvbr16,
                "maskvT": mvT,
            }
        )

    res = run_bass_kernel_spmd(nc, in_maps, list(range(N_CORES)), trace=TRACE)
    LAST_RESULTS = res

    out = np.empty((B, S, D), np.float32)
    for core in range(N_CORES):
        b = core // (N_CORES // B)
        c0 = (core % (N_CORES // B)) * CPC
        out[b, c0 * W:(c0 + CPC) * W, :] = res.results[core]["out"].astype(
            np.float32
        )
    return out

